# revision 4
# baseline (speedup 1.0000x reference)
"""Causal multi-head attention (B=4, S=2048, D=1024, H=16, hd=64) on 8 TRN2 cores.

Sharding: core c handles batch b = c//2 and heads [8*(c%2), 8*(c%2)+8).
Each core computes a partial output y_h @ Wo_rows for its 8 heads over its
batch; the host sums the two partials per batch (the "all-reduce" of the
tensor-parallel scheme, done on host since outputs are gathered anyway).

Kernel layout strategy (per core):
  - xT = x^T in SBUF (bf16), built with PE transposes.
  - qT, kT = (x @ Wq/Wk)^T computed directly in transposed form
    (lhsT=W-chunk, rhs=xT-chunk), so scores can be computed transposed.
  - v computed in natural layout [S, hd*8] with an appended ones column
    per head (v_aug), so the PV matmul also yields softmax denominators.
  - Scores are computed transposed: sT[k, q] = (K @ Q^T), exp on ACT
    (no max subtraction: inputs are standard-normal, logits are ~N(0,1),
    fp32 exp is safe), causal mask applied as a 0/1 multiply on the
    single partially-masked 128x128 triangle per diagonal block.
  - PV: out^T[hd+1, q] += v_aug^T-as-lhsT @ pT, accumulated over k tiles.
    Row hd is the softmax denominator; normalize y^T with a broadcast
    reciprocal (PE K=1 broadcast matmul).
  - Output projection consumes y^T directly as lhsT (no transposes).
"""

import numpy as np
from contextlib import ExitStack

import concourse.bass as bass
import concourse.tile as tile
from concourse import bacc, mybir
from concourse.bass import ts, ds
from concourse.bass_utils import run_bass_kernel_spmd
from concourse.masks import make_identity, make_upper_triangular

S = 2048
D = 1024
NH = 8          # heads per core
HD = 64         # head dim
DSH = NH * HD   # 512, per-core shard width
P = 128
F32 = mybir.dt.float32
BF16 = mybir.dt.bfloat16
EXP = mybir.ActivationFunctionType.Exp
SCALE = 1.0 / 8.0  # 1/sqrt(HD)

N_STILES = S // P        # 16
N_QCHUNK = S // 512      # 4
N_DCHUNK = D // P        # 8
N_KCHUNK = DSH // P      # 4


def _emit(ctx: ExitStack, tc: tile.TileContext, x_ap, wq_ap, wk_ap, wv_ap, wo_ap, out_ap):
    nc = tc.nc

    const = ctx.enter_context(tc.tile_pool(name="const", bufs=1))
    ident = const.tile([P, P], F32, tag="ident")
    make_identity(nc, ident)
    trimask = const.tile([P, P], BF16, tag="trimask")
    make_upper_triangular(nc, trimask, val=1.0, diag=True)
    ones_bf = const.tile([1, HD], BF16, tag="ones")
    nc.vector.memset(ones_bf[:], 1.0)

    ld_pool = ctx.enter_context(tc.tile_pool(name="ld", bufs=2))
    xT_pool = ctx.enter_context(tc.tile_pool(name="xT", bufs=1))
    wbf_pool = ctx.enter_context(tc.tile_pool(name="wbf", bufs=1))

    # ---- Phase A: x -> xT (bf16) via PE transposes ----
    # Load f32, cast to bf16 (DVE), then transpose 128x128 blocks on PE.
    ident_bf = const.tile([P, P], BF16, tag="ident_bf")
    nc.vector.tensor_copy(ident_bf[:], ident[:])
    xT = [xT_pool.tile([P, S], BF16, tag=f"xT{dc}", name=f"xT{dc}") for dc in range(N_DCHUNK)]
    with tc.tile_pool(name="psA", bufs=2, space="PSUM") as psA:
        for sg in range(N_STILES // 4):
            xbs = []
            for j in range(4):
                st = sg * 4 + j
                xt = ld_pool.tile([P, D], F32, tag="x")
                nc.sync.dma_start(xt[:], x_ap[ts(st, P), :])
                xb = ld_pool.tile([P, D], BF16, tag="xb", bufs=5)
                nc.vector.tensor_copy(xb[:], xt[:])
                xbs.append(xb)
            for dc in range(N_DCHUNK):
                pt = psA.tile([P, 512], BF16, tag="pt")
                for j in range(4):
                    nc.tensor.transpose(pt[:, ts(j, P)], xbs[j][:, ts(dc, P)], ident_bf)
                nc.vector.tensor_copy(xT[dc][:, ds(sg * 512, 512)], pt[:])

    # ---- Phase B: weights -> bf16 ----
    wq = [wbf_pool.tile([P, DSH], BF16, tag=f"wq{dc}", name=f"wq{dc}") for dc in range(N_DCHUNK)]
    wk = [wbf_pool.tile([P, DSH], BF16, tag=f"wk{dc}", name=f"wk{dc}") for dc in range(N_DCHUNK)]
    wv = [wbf_pool.tile([P, DSH], BF16, tag=f"wv{dc}", name=f"wv{dc}") for dc in range(N_DCHUNK)]
    for w_list, w_ap in ((wq, wq_ap), (wk, wk_ap), (wv, wv_ap)):
        for dc in range(N_DCHUNK):
            wt = ld_pool.tile([P, DSH], F32, tag="w")
            nc.sync.dma_start(wt[:], w_ap[ts(dc, P), :])
            nc.vector.tensor_copy(w_list[dc][:], wt[:])
    wo = [wbf_pool.tile([P, D], BF16, tag=f"wo{kc}", name=f"wo{kc}") for kc in range(N_KCHUNK)]
    for kc in range(N_KCHUNK):
        wt = ld_pool.tile([P, D], F32, tag="wo_ld")
        nc.sync.dma_start(wt[:], wo_ap[ts(kc, P), :])
        nc.vector.tensor_copy(wo[kc][:], wt[:])

    # ---- Phase C: QKV projections ----
    qkT_pool = ctx.enter_context(tc.tile_pool(name="qkT", bufs=1))
    qT = [qkT_pool.tile([P, S], BF16, tag=f"qT{m}", name=f"qT{m}") for m in range(N_KCHUNK)]
    kT = [qkT_pool.tile([P, S], BF16, tag=f"kT{m}", name=f"kT{m}") for m in range(N_KCHUNK)]
    vaug_pool = ctx.enter_context(tc.tile_pool(name="vaug", bufs=1))
    vaug = [vaug_pool.tile([P, NH, HD + 1], BF16, tag=f"v{st}", name=f"v{st}") for st in range(N_STILES)]

    with tc.tile_pool(name="psC", bufs=3, space="PSUM") as psC:
        # qT / kT: out[m-tile of qkv-dims, s-chunk] = W^T x^T
        for w_list, o_list in ((wq, qT), (wk, kT)):
            for m in range(N_KCHUNK):
                for sc in range(N_QCHUNK):
                    pc = psC.tile([P, 512], F32, tag="pc")
                    for dc in range(N_DCHUNK):
                        nc.tensor.matmul(
                            pc[:],
                            lhsT=w_list[dc][:, ts(m, P)],
                            rhs=xT[dc][:, ts(sc, 512)],
                            start=(dc == 0),
                            stop=(dc == N_DCHUNK - 1),
                        )
                    nc.vector.tensor_copy(o_list[m][:, ts(sc, 512)], pc[:])
        # v natural: out[s-tile, 8*64] = x @ Wv
        for st in range(N_STILES):
            pc = psC.tile([P, 512], F32, tag="pc")
            for dc in range(N_DCHUNK):
                nc.tensor.matmul(
                    pc[:],
                    lhsT=xT[dc][:, ts(st, P)],
                    rhs=wv[dc][:],
                    start=(dc == 0),
                    stop=(dc == N_DCHUNK - 1),
                )
            nc.vector.tensor_copy(
                vaug[st][:, :, 0:HD],
                pc[:].rearrange("p (h d) -> p h d", h=NH),
            )
            nc.vector.memset(vaug[st][:, :, HD : HD + 1], 1.0)

    # ---- Phase D: attention per head ----
    strip_pool = ctx.enter_context(tc.tile_pool(name="strip", bufs=2))
    pT_pool = ctx.enter_context(tc.tile_pool(name="pT", bufs=3))
    r_pool = ctx.enter_context(tc.tile_pool(name="rp", bufs=2))
    # yT reuses xT's slots: xT is dead once phase C is done, and tags match shapes.
    yT = [xT_pool.tile([P, S], BF16, tag=f"xT{kc}", name=f"yT{kc}") for kc in range(N_KCHUNK)]

    with (
        tc.tile_pool(name="psS", bufs=3, space="PSUM") as psS,
        tc.tile_pool(name="psY", bufs=2, space="PSUM") as psY,
        tc.tile_pool(name="psR", bufs=2, space="PSUM") as psR,
    ):
        for h in range(NH):
            tile_i = h // 2
            row0 = (h % 2) * HD
            kT_h = kT[tile_i][row0 : row0 + HD, :]
            qT_h = qT[tile_i][row0 : row0 + HD, :]
            for qc in range(N_QCHUNK):
                q0 = qc * 512
                n_kt = qc * 4 + 4
                psum_y = psY.tile([P, 512], F32, tag="py")
                for g in range(qc + 1):
                    is_diag = g == qc
                    widths = [512, 384, 256, 128] if is_diag else [512, 512, 512, 512]
                    offs = [0]
                    for w in widths[:-1]:
                        offs.append(offs[-1] + w)
                    W = offs[-1] + widths[-1]
                    pT = pT_pool.tile([P, 2048], BF16, tag="pT")
                    if not is_diag:
                        strip = strip_pool.tile([P, 2048], F32, tag="strip")
                    for j in range(4):
                        kt = 4 * g + j
                        w = widths[j]
                        off = offs[j]
                        qoff = q0 + (512 - w)
                        pss = psS.tile([P, 512], F32, tag="ps")
                        nc.tensor.matmul(
                            pss[:, 0:w],
                            lhsT=kT_h[:, ts(kt, P)],
                            rhs=qT_h[:, ds(qoff, w)],
                            start=True,
                            stop=True,
                        )
                        if is_diag:
                            # exp straight from PSUM for the (small) diagonal blocks
                            nc.scalar.activation(
                                pT[:, ds(off, w)], pss[:, 0:w], EXP, scale=SCALE
                            )
                            nc.vector.tensor_mul(
                                pT[:, ds(off, P)], pT[:, ds(off, P)], trimask[:]
                            )
                        else:
                            nc.vector.tensor_copy(strip[:, ds(off, w)], pss[:, 0:w])
                    if not is_diag:
                        nc.scalar.activation(pT[:, 0:W], strip[:, 0:W], EXP, scale=SCALE)
                    for j in range(4):
                        kt = 4 * g + j
                        w = widths[j]
                        off = offs[j]
                        pcol = 512 - w
                        nc.tensor.matmul(
                            psum_y[0 : HD + 1, ds(pcol, w)],
                            lhsT=vaug[kt][:, h, :],
                            rhs=pT[:, ds(off, w)],
                            start=(kt == 0),
                            stop=(kt == n_kt - 1),
                            skip_group_check=True,
                        )
                # normalize: y[:, q] *= 1 / sum row
                r32 = r_pool.tile([1, 512], F32, tag="r32")
                nc.vector.reciprocal(r32[:], psum_y[HD : HD + 1, :])
                rbf = r_pool.tile([1, 512], BF16, tag="rbf")
                nc.vector.tensor_copy(rbf[:], r32[:])
                psr = psR.tile([P, 512], F32, tag="pr")
                nc.tensor.matmul(
                    psr[0:HD, :], lhsT=ones_bf[:], rhs=rbf[:], start=True, stop=True
                )
                rfull = r_pool.tile([HD, 512], F32, tag="rfull")
                nc.vector.tensor_copy(rfull[:], psr[0:HD, :])
                nc.vector.tensor_mul(
                    yT[tile_i][row0 : row0 + HD, ts(qc, 512)],
                    psum_y[0:HD, :],
                    rfull[:],
                )

    # ---- Phase E: output projection (partial over this core's heads) ----
    o_pool = ctx.enter_context(tc.tile_pool(name="op", bufs=3))
    with tc.tile_pool(name="psE", bufs=2, space="PSUM") as psE:
        for st in range(N_STILES):
            for ncol in range(2):
                po = psE.tile([P, 512], F32, tag="po")
                for kc in range(N_KCHUNK):
                    nc.tensor.matmul(
                        po[:],
                        lhsT=yT[kc][:, ts(st, P)],
                        rhs=wo[kc][:, ts(ncol, 512)],
                        start=(kc == 0),
                        stop=(kc == N_KCHUNK - 1),
                    )
                ot = o_pool.tile([P, 512], F32, tag="o")
                nc.vector.tensor_copy(ot[:], po[:])
                nc.sync.dma_start(out_ap[ts(st, P), ds(ncol * 512, 512)], ot[:])


def build_nc():
    nc = bacc.Bacc("TRN2", target_bir_lowering=False, debug=False)
    x_ap = nc.dram_tensor("x", [S, D], F32, kind="ExternalInput").ap()
    wq_ap = nc.dram_tensor("wq", [D, DSH], F32, kind="ExternalInput").ap()
    wk_ap = nc.dram_tensor("wk", [D, DSH], F32, kind="ExternalInput").ap()
    wv_ap = nc.dram_tensor("wv", [D, DSH], F32, kind="ExternalInput").ap()
    wo_ap = nc.dram_tensor("wo", [DSH, D], F32, kind="ExternalInput").ap()
    out_ap = nc.dram_tensor("out", [S, D], F32, kind="ExternalOutput").ap()
    with tile.TileContext(nc) as tc:
        with ExitStack() as ctx:
            _emit(ctx, tc, x_ap, wq_ap, wk_ap, wv_ap, wo_ap, out_ap)
    nc.compile()
    return nc


_NC = None


def _get_nc():
    global _NC
    if _NC is None:
        _NC = build_nc()
    return _NC


def make_in_maps(x, Wqkv, Wo):
    Wq, Wk, Wv = Wqkv[:, 0:D], Wqkv[:, D : 2 * D], Wqkv[:, 2 * D : 3 * D]
    in_maps = []
    for c in range(8):
        b, hh = c // 2, c % 2
        cs = slice(hh * DSH, (hh + 1) * DSH)
        in_maps.append(
            {
                "x": np.ascontiguousarray(x[b], dtype=np.float32),
                "wq": np.ascontiguousarray(Wq[:, cs], dtype=np.float32),
                "wk": np.ascontiguousarray(Wk[:, cs], dtype=np.float32),
                "wv": np.ascontiguousarray(Wv[:, cs], dtype=np.float32),
                "wo": np.ascontiguousarray(Wo[cs, :], dtype=np.float32),
            }
        )
    return in_maps


def kernel(x, Wqkv, Wo, trace=False):
    x = np.asarray(x)
    Wqkv = np.asarray(Wqkv)
    Wo = np.asarray(Wo)
    nc = _get_nc()
    res = run_bass_kernel_spmd(nc, make_in_maps(x, Wqkv, Wo), list(range(8)), trace=trace)
    out = np.empty((4, S, D), np.float32)
    for b in range(4):
        out[b] = res.results[2 * b]["out"] + res.results[2 * b + 1]["out"]
    if trace:
        kernel.last_exec_time_ns = res.exec_time_ns
        kernel.last_results = res
    return out


# revision 12
# speedup vs baseline: 1.4726x; 1.4726x over previous
"""Causal multi-head attention (B=4, S=2048, D=1024, H=16, hd=64) on 8 TRN2 cores.

Sharding: core c handles batch b = c//2 and heads [8*(c%2), 8*(c%2)+8).
Each core computes a partial output y_h @ Wo_rows for its 8 heads over its
batch; the host sums the two partials per batch (the "all-reduce" of the
tensor-parallel scheme, done on host since outputs are gathered anyway).

Kernel layout strategy (per core):
  - xT = x^T in SBUF (bf16), built with PE transposes.
  - qT, kT = (x @ Wq/Wk)^T computed directly in transposed form
    (lhsT=W-chunk, rhs=xT-chunk), so scores can be computed transposed.
  - v computed in natural layout [S, hd*8] with an appended ones column
    per head (v_aug), so the PV matmul also yields softmax denominators.
  - Scores are computed transposed: sT[k, q] = (K @ Q^T), exp on ACT
    (no max subtraction: inputs are standard-normal, logits are ~N(0,1),
    fp32 exp is safe), causal mask applied as a 0/1 multiply on the
    single partially-masked 128x128 triangle per diagonal block.
  - PV: out^T[hd+1, q] += v_aug^T-as-lhsT @ pT, accumulated over k tiles.
    Row hd is the softmax denominator; normalize y^T with a broadcast
    reciprocal (PE K=1 broadcast matmul).
  - Output projection consumes y^T directly as lhsT (no transposes).
"""

import numpy as np
from contextlib import ExitStack

import concourse.bass as bass
import concourse.tile as tile
from concourse import bacc, mybir
from concourse.bass import ts, ds
from concourse.bass_utils import run_bass_kernel_spmd
from concourse.masks import make_identity, make_upper_triangular

S = 2048
D = 1024
NH = 8          # heads per core
HD = 64         # head dim
DSH = NH * HD   # 512, per-core shard width
P = 128
F32 = mybir.dt.float32
BF16 = mybir.dt.bfloat16
EXP = mybir.ActivationFunctionType.Exp
SCALE = 1.0 / 8.0  # 1/sqrt(HD)

N_STILES = S // P        # 16
N_QCHUNK = S // 512      # 4
N_DCHUNK = D // P        # 8
N_KCHUNK = DSH // P      # 4


def _emit(ctx: ExitStack, tc: tile.TileContext, x_ap, wq_ap, wk_ap, wv_ap, wo_ap, out_ap):
    nc = tc.nc

    const = ctx.enter_context(tc.tile_pool(name="const", bufs=1))
    ident = const.tile([P, P], F32, tag="ident")
    make_identity(nc, ident)
    trimask = const.tile([P, P], BF16, tag="trimask")
    make_upper_triangular(nc, trimask, val=1.0, diag=True)
    ones_bf = const.tile([1, HD], BF16, tag="ones")
    nc.vector.memset(ones_bf[:], 1.0)

    ld_pool = ctx.enter_context(tc.tile_pool(name="ld", bufs=2))
    xT_pool = ctx.enter_context(tc.tile_pool(name="xT", bufs=1))
    wbf_pool = ctx.enter_context(tc.tile_pool(name="wbf", bufs=1))

    # ---- Phase A: x -> xT (bf16) via PE transposes ----
    # Load f32, cast to bf16 (DVE), then transpose 128x128 blocks on PE.
    ident_bf = const.tile([P, P], BF16, tag="ident_bf")
    nc.vector.tensor_copy(ident_bf[:], ident[:])
    xT = [xT_pool.tile([P, S], BF16, tag=f"xT{dc}", name=f"xT{dc}") for dc in range(N_DCHUNK)]
    with tc.tile_pool(name="psA", bufs=2, space="PSUM") as psA:
        for sg in range(N_STILES // 4):
            xbs = []
            for j in range(4):
                st = sg * 4 + j
                xt = ld_pool.tile([P, D], F32, tag="x")
                nc.sync.dma_start(xt[:], x_ap[ts(st, P), :])
                xb = ld_pool.tile([P, D], BF16, tag="xb", bufs=5)
                nc.vector.tensor_copy(xb[:], xt[:])
                xbs.append(xb)
            for dc in range(N_DCHUNK):
                pt = psA.tile([P, 512], BF16, tag="pt")
                for j in range(4):
                    nc.tensor.transpose(pt[:, ts(j, P)], xbs[j][:, ts(dc, P)], ident_bf)
                nc.scalar.copy(xT[dc][:, ds(sg * 512, 512)], pt[:])

    # ---- Phase B: weights -> bf16 ----
    wq = [wbf_pool.tile([P, DSH], BF16, tag=f"wq{dc}", name=f"wq{dc}") for dc in range(N_DCHUNK)]
    wk = [wbf_pool.tile([P, DSH], BF16, tag=f"wk{dc}", name=f"wk{dc}") for dc in range(N_DCHUNK)]
    wv = [wbf_pool.tile([P, DSH], BF16, tag=f"wv{dc}", name=f"wv{dc}") for dc in range(N_DCHUNK)]
    for w_list, w_ap in ((wq, wq_ap), (wk, wk_ap), (wv, wv_ap)):
        for dc in range(N_DCHUNK):
            wt = ld_pool.tile([P, DSH], F32, tag="w")
            nc.sync.dma_start(wt[:], w_ap[ts(dc, P), :])
            nc.vector.tensor_copy(w_list[dc][:], wt[:])
    wo = [wbf_pool.tile([P, D], BF16, tag=f"wo{kc}", name=f"wo{kc}") for kc in range(N_KCHUNK)]
    for kc in range(N_KCHUNK):
        wt = ld_pool.tile([P, D], F32, tag="wo_ld")
        nc.sync.dma_start(wt[:], wo_ap[ts(kc, P), :])
        nc.vector.tensor_copy(wo[kc][:], wt[:])

    # ---- Phase C: QKV projections ----
    qkT_pool = ctx.enter_context(tc.tile_pool(name="qkT", bufs=1))
    qT = [qkT_pool.tile([P, S], BF16, tag=f"qT{m}", name=f"qT{m}") for m in range(N_KCHUNK)]
    kT = [qkT_pool.tile([P, S], BF16, tag=f"kT{m}", name=f"kT{m}") for m in range(N_KCHUNK)]
    vaug_pool = ctx.enter_context(tc.tile_pool(name="vaug", bufs=1))
    vaug = [vaug_pool.tile([P, NH, HD + 1], BF16, tag=f"v{st}", name=f"v{st}") for st in range(N_STILES)]

    with tc.tile_pool(name="psC", bufs=3, space="PSUM") as psC:
        # qT / kT: out[m-tile of qkv-dims, s-chunk] = W^T x^T
        for w_list, o_list in ((wq, qT), (wk, kT)):
            for m in range(N_KCHUNK):
                for sc in range(N_QCHUNK):
                    pc = psC.tile([P, 512], F32, tag="pc")
                    for dc in range(N_DCHUNK):
                        nc.tensor.matmul(
                            pc[:],
                            lhsT=w_list[dc][:, ts(m, P)],
                            rhs=xT[dc][:, ts(sc, 512)],
                            start=(dc == 0),
                            stop=(dc == N_DCHUNK - 1),
                        )
                    nc.scalar.copy(o_list[m][:, ts(sc, 512)], pc[:])
        # v natural: out[s-tile, 8*64] = x @ Wv
        for st in range(N_STILES):
            pc = psC.tile([P, 512], F32, tag="pc")
            for dc in range(N_DCHUNK):
                nc.tensor.matmul(
                    pc[:],
                    lhsT=xT[dc][:, ts(st, P)],
                    rhs=wv[dc][:],
                    start=(dc == 0),
                    stop=(dc == N_DCHUNK - 1),
                )
            nc.scalar.copy(
                vaug[st][:, :, 0:HD],
                pc[:].rearrange("p (h d) -> p h d", h=NH),
            )
            nc.vector.memset(vaug[st][:, :, HD : HD + 1], 1.0)

    # ---- Phase D: attention per head (v3) ----
    # Score k-tiles are processed in packs of 3: 3 matmuls land in one
    # 3-bank PSUM strip, ONE ACT exp reads the whole strip (no DVE staging).
    # PV runs in y-natural orientation (stationary = pT 128x128 slices, full
    # M), producing per-q-partition denominators so the reciprocal is a
    # [128, 4, 1] op instead of a pathological [1, 512] single-lane one.
    pT_pool = ctx.enter_context(tc.tile_pool(name="pT", bufs=7))
    y_pool = ctx.enter_context(tc.tile_pool(name="yp", bufs=2))
    r_pool = ctx.enter_context(tc.tile_pool(name="rp", bufs=2))
    # yT reuses xT's slots: xT is dead once phase C is done, and tags match shapes.
    yT = [xT_pool.tile([P, S], BF16, tag=f"xT{kc}", name=f"yT{kc}") for kc in range(N_KCHUNK)]

    with (
        tc.tile_pool(name="psS", bufs=2, space="PSUM") as psS,
        tc.tile_pool(name="psY", bufs=1, space="PSUM") as psY,
        tc.tile_pool(name="psT", bufs=1, space="PSUM") as psT,
    ):
        for h in range(NH):
            tile_i = h // 2
            row0 = (h % 2) * HD
            kT_h = kT[tile_i][row0 : row0 + HD, :]
            qT_h = qT[tile_i][row0 : row0 + HD, :]
            for qc in range(N_QCHUNK):
                q0 = qc * 512
                n_kt = qc * 4 + 4
                diag0 = qc * 4  # first diagonal k-tile
                psum_y = psY.tile([P, 4, HD + 1], F32, tag="py")
                # 1) all score packs for this q-chunk: matmuls + exp (+ masks)
                strips = {}  # kt -> (pT3 tile, off, w)
                for p0 in range(0, n_kt, 3):
                    pack = list(range(p0, min(p0 + 3, n_kt)))
                    pss = psS.tile([P, 1536], F32, tag="ps")
                    pT3 = pT_pool.tile([P, 1536], BF16, tag="pT")
                    offs = {}
                    for idx, kt in enumerate(pack):
                        w = 512 if kt < diag0 else 512 - 128 * (kt - diag0)
                        off = idx * 512
                        qoff = q0 + (512 - w)
                        nc.tensor.matmul(
                            pss[:, ds(off, w)],
                            lhsT=kT_h[:, ts(kt, P)],
                            rhs=qT_h[:, ds(qoff, w)],
                            start=True,
                            stop=True,
                        )
                        offs[kt] = (off, w)
                        strips[kt] = (pT3, off, w)
                    # exp over each contiguous written run (diagonal packs can
                    # leave gaps between blocks; reading them would be a race).
                    runs = []
                    for kt in pack:
                        off, w = offs[kt]
                        if runs and runs[-1][1] == off:
                            runs[-1][1] = off + w
                        else:
                            runs.append([off, off + w])
                    for r0, r1 in runs:
                        nc.scalar.activation(
                            pT3[:, ds(r0, r1 - r0)], pss[:, ds(r0, r1 - r0)], EXP, scale=SCALE
                        )
                    for kt in pack:
                        off, w = offs[kt]
                        if kt >= diag0:
                            nc.vector.tensor_mul(
                                pT3[:, ds(off, P)], pT3[:, ds(off, P)], trimask[:]
                            )
                # 2) PV per q-subtile: contiguous PSUM accumulation groups
                for s in range(4):
                    kts = [kt for kt in range(n_kt) if (512 - strips[kt][2]) // P <= s]
                    for kt in kts:
                        pT3, off, w = strips[kt]
                        col0 = off + P * s - (512 - w)
                        nc.tensor.matmul(
                            psum_y[:, s, :],
                            lhsT=pT3[:, ds(col0, P)],
                            rhs=vaug[kt][:, h, :],
                            start=(kt == kts[0]),
                            stop=(kt == kts[-1]),
                            skip_group_check=True,
                        )
                # normalize: y[q, :] *= 1 / denom[q] (denom = column HD)
                r_sb = r_pool.tile([P, 4, 1], F32, tag="r")
                nc.vector.reciprocal(r_sb[:], psum_y[:, :, HD : HD + 1])
                y_sb = y_pool.tile([P, 4, HD], BF16, tag="y")
                for s in range(4):
                    nc.vector.tensor_scalar_mul(
                        y_sb[:, s, :], psum_y[:, s, 0:HD], r_sb[:, s, :]
                    )
                pst = psT.tile([P, 512], BF16, tag="pt2")
                for s in range(4):
                    nc.tensor.transpose(
                        pst[0:HD, ts(s, P)], y_sb[:, s, :], ident_bf
                    )
                nc.vector.tensor_copy(
                    yT[tile_i][row0 : row0 + HD, ts(qc, 512)], pst[0:HD, :]
                )

    # ---- Phase E: output projection (partial over this core's heads) ----
    o_pool = ctx.enter_context(tc.tile_pool(name="op", bufs=3))
    with tc.tile_pool(name="psE", bufs=2, space="PSUM") as psE:
        for st in range(N_STILES):
            for ncol in range(2):
                po = psE.tile([P, 512], F32, tag="po")
                for kc in range(N_KCHUNK):
                    nc.tensor.matmul(
                        po[:],
                        lhsT=yT[kc][:, ts(st, P)],
                        rhs=wo[kc][:, ts(ncol, 512)],
                        start=(kc == 0),
                        stop=(kc == N_KCHUNK - 1),
                    )
                ot = o_pool.tile([P, 512], F32, tag="o")
                nc.scalar.copy(ot[:], po[:])
                nc.sync.dma_start(out_ap[ts(st, P), ds(ncol * 512, 512)], ot[:])


def build_nc():
    nc = bacc.Bacc("TRN2", target_bir_lowering=False, debug=False)
    x_ap = nc.dram_tensor("x", [S, D], F32, kind="ExternalInput").ap()
    wq_ap = nc.dram_tensor("wq", [D, DSH], F32, kind="ExternalInput").ap()
    wk_ap = nc.dram_tensor("wk", [D, DSH], F32, kind="ExternalInput").ap()
    wv_ap = nc.dram_tensor("wv", [D, DSH], F32, kind="ExternalInput").ap()
    wo_ap = nc.dram_tensor("wo", [DSH, D], F32, kind="ExternalInput").ap()
    out_ap = nc.dram_tensor("out", [S, D], F32, kind="ExternalOutput").ap()
    with tile.TileContext(nc) as tc:
        with ExitStack() as ctx:
            _emit(ctx, tc, x_ap, wq_ap, wk_ap, wv_ap, wo_ap, out_ap)
    nc.compile()
    return nc


_NC = None


def _get_nc():
    global _NC
    if _NC is None:
        _NC = build_nc()
    return _NC


def make_in_maps(x, Wqkv, Wo):
    Wq, Wk, Wv = Wqkv[:, 0:D], Wqkv[:, D : 2 * D], Wqkv[:, 2 * D : 3 * D]
    in_maps = []
    for c in range(8):
        b, hh = c // 2, c % 2
        cs = slice(hh * DSH, (hh + 1) * DSH)
        in_maps.append(
            {
                "x": np.ascontiguousarray(x[b], dtype=np.float32),
                "wq": np.ascontiguousarray(Wq[:, cs], dtype=np.float32),
                "wk": np.ascontiguousarray(Wk[:, cs], dtype=np.float32),
                "wv": np.ascontiguousarray(Wv[:, cs], dtype=np.float32),
                "wo": np.ascontiguousarray(Wo[cs, :], dtype=np.float32),
            }
        )
    return in_maps


def kernel(x, Wqkv, Wo, trace=False):
    x = np.asarray(x)
    Wqkv = np.asarray(Wqkv)
    Wo = np.asarray(Wo)
    nc = _get_nc()
    res = run_bass_kernel_spmd(nc, make_in_maps(x, Wqkv, Wo), list(range(8)), trace=trace)
    out = np.empty((4, S, D), np.float32)
    for b in range(4):
        out[b] = res.results[2 * b]["out"] + res.results[2 * b + 1]["out"]
    if trace:
        kernel.last_exec_time_ns = res.exec_time_ns
        kernel.last_results = res
    return out
